# revision 7
# baseline (speedup 1.0000x reference)
"""LIF (leaky integrate-and-fire) forward kernel for Trainium2, 8 NeuronCores.

Recurrence (per element of [B, N], serial over T):
    v_t = DECAY * (v_{t-1} * (1 - s_{t-1})) + x_t      (REST = 0)
    s_t = (v_t > THRESHOLD)

State formulation with w_t = v_t * [v_t <= THRESHOLD] (post-reset membrane):
    v_t = (w_{t-1} * DECAY) + x_t        -> one scalar_tensor_tensor
    w_t = (v_t is_le THR) * v_t          -> one scalar_tensor_tensor

Optimizations over the fp32 baseline (166 us):
  * whole chain in fp16 (input DMA halved; DVE 2-byte ops run 2x).
    Verified vs the fp32 oracle on the real seed-0 inputs: ~2.2e3 of 23.6e6
    spikes flip (rel err ~9.6e-3 < 2e-2 gate), including worst-case
    double-rounding inside the STT.
  * recurrence column-split between DVE (vector) and Pool (gpsimd) engines.
  * spike output bit-packed on the tensor engine: PE accumulates
    sign_t * 4^(t%8) into PSUM over 8-step groups via scaled-identity
    matmuls (digits {-1,0,1} shifted to {0,1,2}: base-4, uniquely
    decodable).  Output: 4 int16 planes instead of 32 fp8 planes
    (8 MiB -> 2 MiB per core).  Host decodes bits.
  * Act engine computes Sign(v - THR) for most columns (bf16 {-1,0,1});
    DVE covers the rest with is_gt ({0,1} -> same decode after offset).

Sharding: batch dim (128) split 16 rows/core across 8 cores; per-core,
per-step slab is a contiguous 512 KiB fp16 block viewed as [128, 2048].
"""

import numpy as np
import ml_dtypes

import concourse.bacc as bacc
import concourse.mybir as mybir
from concourse.tile import TileContext
from concourse.bass_utils import run_bass_kernel_spmd

T, B, N = 32, 128, 16384
N_CORES = 8
B_SH = B // N_CORES          # 16 batch rows per core
S = B_SH * N                 # 262144 elements per core per time step
P = 128                      # SBUF partitions
F = S // P                   # 2048 free-dim elements
DECAY = 0.2
THR = 0.3

GP = 8                       # time steps per pack group
NG = T // GP                 # pack groups
OFFSET = sum(4 ** k for k in range(GP))   # 21845: digit shift {-1,0,1}->{0,1,2}

# column ownership (tuned against the engine cost model, then the trace)
# note: GPSIMD/Pool has no elementwise ALU on TRN2 — recurrence lives on DVE
GS = 0                       # DVE computes spikes via is_gt on cols [0, GS);
                             # Act does Sign on cols [GS, F)
MMC = 512                    # matmul / psum column chunk (one PSUM bank)

TRACE = False                # set True (e.g. from test.py) to capture a profile

_BUILT = {}


def _build_nc():
    nc = bacc.Bacc("TRN2", debug=False, num_devices=N_CORES)
    f32 = mybir.dt.float32
    f16 = mybir.dt.float16
    bf16 = mybir.dt.bfloat16
    i16 = mybir.dt.int16
    Alu = mybir.AluOpType
    Act = mybir.ActivationFunctionType

    x = nc.dram_tensor("x", [T, S], f16, kind="ExternalInput").ap()
    wq = nc.dram_tensor("wq", [P, GP * P], bf16, kind="ExternalInput").ap()
    y = nc.dram_tensor("y", [NG, S], i16, kind="ExternalOutput").ap()
    xr = x.rearrange("t (p f) -> t p f", p=P)
    yr = y.rearrange("g (p f) -> g p f", p=P)

    with TileContext(nc) as tc:
        with (
            tc.tile_pool(name="state", bufs=1) as state_pool,
            tc.tile_pool(name="const", bufs=1) as const_pool,
            tc.tile_pool(name="xin", bufs=6) as xin_pool,
            tc.tile_pool(name="vtmp", bufs=3) as v_pool,
            tc.tile_pool(name="sgn", bufs=3) as s_pool,
            tc.tile_pool(name="evac", bufs=2) as e_pool,
            tc.tile_pool(name="pack", bufs=2, space="PSUM") as psum_pool,
        ):
            negthr = nc.alloc_sbuf_tensor("const_negthr", [P, 1], f32).ap()
            nc.gpsimd.memset(negthr, -THR)
            wq_s = const_pool.tile([P, GP * P], bf16)
            nc.sync.dma_start(out=wq_s[:], in_=wq)

            w = state_pool.tile([P, F], f16)
            psum = None
            pending = None      # deferred (group, psum_tile) evacuation

            def emit_evac(g, ps):
                ev = e_pool.tile([P, F], i16)
                # fp32 psum (exact ints <= 21845) -> int16 sbuf -> HBM
                # (GPSIMD has no PSUM port; DVE is the bottleneck -> Act)
                nc.scalar.copy(out=ev[:], in_=ps[:])
                nc.sync.dma_start(out=yr[g], in_=ev[:])

            for t in range(T):
                g, k = divmod(t, GP)
                xt = xin_pool.tile([P, F], f16)
                nc.sync.dma_start(out=xt[:], in_=xr[t])

                if t == 0:
                    vt = xt      # w_{-1} = 0 so v_0 = x_0
                else:
                    vt = v_pool.tile([P, F], f16)
                    # v = (w * DECAY) + x
                    nc.vector.scalar_tensor_tensor(
                        out=vt[:], in0=w[:], scalar=DECAY,
                        in1=xt[:], op0=Alu.mult, op1=Alu.add,
                    )

                st = s_pool.tile([P, F], bf16)
                if GS:
                    nc.vector.tensor_scalar(
                        out=st[:, :GS], in0=vt[:, :GS], scalar1=THR,
                        scalar2=None, op0=Alu.is_gt,
                    )
                nc.scalar.activation(st[:, GS:], vt[:, GS:], Act.Sign,
                                     bias=negthr)

                if t < T - 1:
                    # w = (v is_le THR) * v
                    nc.vector.scalar_tensor_tensor(
                        out=w[:], in0=vt[:], scalar=THR,
                        in1=vt[:], op0=Alu.is_le, op1=Alu.mult,
                    )

                if k == 0:
                    psum = psum_pool.tile([P, F], f32)
                lhsT = wq_s[:, k * P:(k + 1) * P]
                for c in range(0, F, MMC):
                    nc.tensor.matmul(
                        psum[:, c:c + MMC], lhsT, st[:, c:c + MMC],
                        start=(k == 0), stop=(k == GP - 1),
                    )

                if pending is not None and k >= 2:
                    emit_evac(*pending)
                    pending = None
                if k == GP - 1:
                    pending = (g, psum)
            emit_evac(*pending)
    nc.compile()
    return nc


def _pack_weights() -> np.ndarray:
    wq = np.zeros((P, GP, P), np.float32)
    idx = np.arange(P)
    for k in range(GP):
        wq[idx, k, idx] = 4.0 ** k
    return wq.reshape(P, GP * P).astype(ml_dtypes.bfloat16)


LAST_RESULTS = None


def kernel(tx):
    global LAST_RESULTS
    tx = np.asarray(tx)
    assert tx.shape == (T, B, N) and tx.dtype == np.float32

    if "nc" not in _BUILT:
        _BUILT["nc"] = _build_nc()
    nc = _BUILT["nc"]

    tx16 = tx.astype(np.float16)
    wq = _pack_weights()
    in_maps = [
        {
            "x": np.ascontiguousarray(
                tx16[:, c * B_SH:(c + 1) * B_SH, :]).reshape(T, S),
            "wq": wq,
        }
        for c in range(N_CORES)
    ]
    res = run_bass_kernel_spmd(nc, in_maps, core_ids=list(range(N_CORES)),
                               trace=TRACE)
    LAST_RESULTS = res

    out = np.empty((T, B, N), dtype=np.float32)
    for c in range(N_CORES):
        packed = np.asarray(res.results[c]["y"]).astype(np.int32) + OFFSET
        packed = packed.reshape(NG, B_SH, N)
        for g in range(NG):
            for k in range(GP):
                dig = (packed[g] >> (2 * k)) & 3
                out[g * GP + k, c * B_SH:(c + 1) * B_SH, :] = (dig == 2)
    return out


# revision 17
# speedup vs baseline: 1.1253x; 1.1253x over previous
"""LIF (leaky integrate-and-fire) forward kernel for Trainium2, 8 NeuronCores.

Reference recurrence (per element of [B, N], serial over T):
    v_t = DECAY * (v_{t-1} * (1 - s_{t-1})) + x_t      (REST = 0)
    s_t = (v_t > THRESHOLD)

Scaled-state formulation.  With c_t a per-step compile-time scale where
c_t = c_{t-1} / DECAY (up to exact power-of-2 renormalizations), and
q_t := c_t * v_t, x'_t := c_t * x_t (host pre-scales):

    q_{t+1} = [q_t <= c_t*THR] * q_t * rho_{t+1} + x'_{t+1}

The DECAY multiply vanishes into the scale schedule; renormalization every
5 steps (exact powers of two) keeps q in fp16 range (|q| < 5e3).  Verified
bit-accurately on the seed-0 inputs: ~2.2e3 of 23.6e6 spikes flip
(rel err 9.6e-3 < 2e-2 gate).

Engine mapping (driven by measured TRN2 ISA constraints — DVE
scalar_tensor_tensor has no fast mode, tensor_scalar runs 4x and
tensor_tensor 2x on all-fp16 operands; GPSIMD has no ALU / PSUM port):
  * the whole non-renorm step is ONE DVE instruction: scalar_tensor_tensor
    with acc=AddAccumulate onto the DMA-prefetched x'_{t+1} tile:
        X_{t+1} += (q_t is_le thr_t) * q_t
    Input DMA stays off the serial chain; the chain is just 31 DVE ops.
  * renorm steps use tensor_scalar (mask {0,rho}, 4x) + tensor_tensor
    (r = mask*q, 2x) + tensor_tensor (X += r via in-place add, 2x).
  * Act computes Sign(q_t - thr_t) -> fp16 {-1,0,1} each step as the spike
    indicator; the tensor engine packs it over 8-step groups as
    sum_k sign_k*4^k into PSUM via scaled-identity matmuls; Act evacuates
    PSUM -> int16 (output 8 MiB -> 2 MiB per core).  Host decodes digits
    (spike <=> digit+1 == 2).

Sharding: batch dim (128) split 16 rows/core across 8 cores; per-core,
per-step slab is a contiguous 512 KiB fp16 block viewed as [128, 2048].
"""

import numpy as np

import concourse.bacc as bacc
import concourse.mybir as mybir
from concourse.tile import TileContext
from concourse.bass_utils import run_bass_kernel_spmd

T, B, N = 32, 128, 16384
N_CORES = 8
B_SH = B // N_CORES          # 16 batch rows per core
S = B_SH * N                 # 262144 elements per core per time step
P = 128                      # SBUF partitions
F = S // P                   # 2048 free-dim elements
DECAY = 0.2
THR = 0.3

GP = 8                       # time steps per pack group
NG = T // GP                 # pack groups
OFFSET = sum(4 ** k for k in range(GP))   # 21845: digit shift into {0,1,2}

# scale schedule: c[t] = c[t-1]*5, renormalized by exact 2^-e at steps in ES
ES = {5: 11, 10: 12, 15: 12, 20: 11, 25: 12, 30: 12}
C_SCHED = [1.0]
for _t in range(1, T):
    _c = C_SCHED[-1] * 5.0
    if _t in ES:
        _c *= 2.0 ** -ES[_t]
    C_SCHED.append(_c)
RHO = {_t - 1: 2.0 ** -ES[_t] for _t in ES}       # renorm factor used at step t
THR_T = [float(np.float32(THR * c)) for c in C_SCHED]
SPECIAL = sorted(RHO) + [T - 1]                   # pack fed by Act Sign

MMC = 512                    # matmul column chunk (one PSUM bank)

TRACE = False                # set True (e.g. from test.py) to capture a profile

_BUILT = {}


def _build_nc():
    nc = bacc.Bacc("TRN2", debug=False, num_devices=N_CORES)
    f32 = mybir.dt.float32
    f16 = mybir.dt.float16
    i16 = mybir.dt.int16
    Alu = mybir.AluOpType
    Act = mybir.ActivationFunctionType

    x = nc.dram_tensor("x", [T, S], f16, kind="ExternalInput").ap()
    wq = nc.dram_tensor("wq", [P, GP * P], f16, kind="ExternalInput").ap()
    y = nc.dram_tensor("y", [NG, S], i16, kind="ExternalOutput").ap()
    xr = x.rearrange("t (p f) -> t p f", p=P)
    yr = y.rearrange("g (p f) -> g p f", p=P)

    def stt_acc(out, in0, scalar, in1, op0, op1):
        # out += (in0 op0 scalar) op1 in1 — DVE dest-accumulation variant of
        # scalar_tensor_tensor (not exposed by the bass wrapper)
        eng = nc.vector
        return eng.add_instruction(
            mybir.InstTensorScalarPtr(
                name=nc.get_next_instruction_name(),
                is_scalar_tensor_tensor=True,
                op0=op0,
                op1=op1,
                acc="AddAccumulate",
                ins=[
                    eng.lower_ap(in0),
                    eng.lower_ap_or_imm(float(scalar)),
                    eng.lower_ap(in1),
                ],
                outs=[eng.lower_ap(out)],
            )
        )

    with TileContext(nc) as tc:
        with (
            tc.tile_pool(name="const", bufs=1) as const_pool,
            tc.tile_pool(name="qin", bufs=8) as q_pool,
            tc.tile_pool(name="mask", bufs=2) as m_pool,
            tc.tile_pool(name="sgn", bufs=3) as s_pool,
            tc.tile_pool(name="evac", bufs=2) as e_pool,
            tc.tile_pool(name="pack", bufs=2, space="PSUM") as psum_pool,
        ):
            wq_s = const_pool.tile([P, GP * P], f16)
            nc.sync.dma_start(out=wq_s[:], in_=wq)
            negthr = {}
            for t in SPECIAL:
                ap = nc.alloc_sbuf_tensor(f"const_negthr{t}", [P, 1], f32).ap()
                nc.gpsimd.memset(ap, -THR_T[t])
                negthr[t] = ap

            psum = None
            pending = None      # deferred (group, psum_tile) evacuation

            def emit_evac(g, ps):
                ev = e_pool.tile([P, F], i16)
                # fp32 psum (exact ints, |.| <= 21845) -> int16 -> HBM
                nc.scalar.copy(out=ev[:], in_=ps[:])
                nc.sync.dma_start(out=yr[g], in_=ev[:])

            qt = q_pool.tile([P, F], f16)
            nc.sync.dma_start(out=qt[:], in_=xr[0])
            for t in range(T):
                g, k = divmod(t, GP)

                if t in SPECIAL:
                    # renorm/final step: mask digits unusable -> Act Sign
                    feed = s_pool.tile([P, F], f16)
                    nc.scalar.activation(feed[:], qt[:], Act.Sign,
                                         bias=negthr[t])

                if t < T - 1:
                    q_next = q_pool.tile([P, F], f16)
                    nc.sync.dma_start(out=q_next[:], in_=xr[t + 1])
                    # mask = [q <= thr_t] * rho   (4x tensor_scalar)
                    mt = m_pool.tile([P, F], f16)
                    nc.vector.tensor_scalar(
                        out=mt[:], in0=qt[:], scalar1=THR_T[t],
                        scalar2=RHO.get(t, 1.0), op0=Alu.is_le, op1=Alu.mult,
                    )
                    if t not in SPECIAL:
                        feed = mt     # {0,1}: digit 0 <=> spike
                    # r = mask * q ; X_{t+1} += r   (2x tensor_tensors)
                    rt = m_pool.tile([P, F], f16)
                    nc.vector.tensor_tensor(
                        out=rt[:], in0=mt[:], in1=qt[:], op=Alu.mult)
                    nc.vector.tensor_tensor(
                        out=q_next[:], in0=rt[:], in1=q_next[:],
                        op=Alu.add)
                    qt = q_next

                if k == 0:
                    psum = psum_pool.tile([P, F], f32)
                lhsT = wq_s[:, k * P:(k + 1) * P]
                for c in range(0, F, MMC):
                    nc.tensor.matmul(
                        psum[:, c:c + MMC], lhsT, feed[:, c:c + MMC],
                        start=(k == 0), stop=(k == GP - 1),
                    )

                if pending is not None and k >= 2:
                    emit_evac(*pending)
                    pending = None
                if k == GP - 1:
                    pending = (g, psum)
            emit_evac(*pending)
    nc.compile()
    return nc


def _pack_weights() -> np.ndarray:
    w = np.zeros((P, GP, P), np.float32)
    idx = np.arange(P)
    for k in range(GP):
        w[idx, k, idx] = 4.0 ** k
    return w.reshape(P, GP * P).astype(np.float16)


LAST_RESULTS = None


def kernel(tx):
    global LAST_RESULTS
    tx = np.asarray(tx)
    assert tx.shape == (T, B, N) and tx.dtype == np.float32

    if "nc" not in _BUILT:
        _BUILT["nc"] = _build_nc()
    nc = _BUILT["nc"]

    xs = np.empty((T, B, N), np.float16)
    for t in range(T):
        xs[t] = (tx[t] * np.float32(C_SCHED[t])).astype(np.float16)
    wq = _pack_weights()
    in_maps = [
        {
            "x": np.ascontiguousarray(
                xs[:, c * B_SH:(c + 1) * B_SH, :]).reshape(T, S),
            "wq": wq,
        }
        for c in range(N_CORES)
    ]
    res = run_bass_kernel_spmd(nc, in_maps, core_ids=list(range(N_CORES)),
                               trace=TRACE)
    LAST_RESULTS = res

    out = np.empty((T, B, N), dtype=np.float32)
    for c in range(N_CORES):
        packed = np.asarray(res.results[c]["y"]).astype(np.int32) + OFFSET
        packed = packed.reshape(NG, B_SH, N)
        for t in range(T):
            g, k = divmod(t, GP)
            dig = (packed[g] >> (2 * k)) & 3
            # special steps: Sign digits {-1,0,1}+1 -> spike == 2;
            # normal steps: mask digits {0,1}+1 -> spike == NOT mask == 1
            want = 2 if t in SPECIAL else 1
            out[t, c * B_SH:(c + 1) * B_SH, :] = (dig == want)
    return out


# revision 19
# speedup vs baseline: 1.3369x; 1.1880x over previous
"""LIF (leaky integrate-and-fire) forward kernel for Trainium2, 8 NeuronCores.

Reference recurrence (per element of [B, N], serial over T):
    v_t = DECAY * (v_{t-1} * (1 - s_{t-1})) + x_t      (REST = 0)
    s_t = (v_t > THRESHOLD)

Scaled-state formulation.  With c_t a per-step compile-time scale where
c_t = c_{t-1} / DECAY (up to exact power-of-2 renormalizations), and
q_t := c_t * v_t, x'_t := c_t * x_t (host pre-scales):

    q_{t+1} = [q_t <= c_t*THR] * q_t * rho_{t+1} + x'_{t+1}

The DECAY multiply vanishes into the scale schedule; renormalization every
5 steps (exact powers of two) keeps q in fp16 range (|q| < 5e3).  Verified
bit-accurately on the seed-0 inputs: ~2.2e3 of 23.6e6 spikes flip
(rel err 9.6e-3 < 2e-2 gate).

Engine mapping (driven by measured TRN2 ISA constraints — DVE
scalar_tensor_tensor has no fast mode, tensor_scalar runs 4x and
tensor_tensor 2x on all-fp16 operands; GPSIMD has no ALU / PSUM port):
  * the whole non-renorm step is ONE DVE instruction: scalar_tensor_tensor
    with acc=AddAccumulate onto the DMA-prefetched x'_{t+1} tile:
        X_{t+1} += (q_t is_le thr_t) * q_t
    Input DMA stays off the serial chain; the chain is just 31 DVE ops.
  * renorm steps use tensor_scalar (mask {0,rho}, 4x) + tensor_tensor
    (r = mask*q, 2x) + tensor_tensor (X += r via in-place add, 2x).
  * Act computes Sign(q_t - thr_t) -> fp16 {-1,0,1} each step as the spike
    indicator; the tensor engine packs it over 8-step groups as
    sum_k sign_k*4^k into PSUM via scaled-identity matmuls; Act evacuates
    PSUM -> int16 (output 8 MiB -> 2 MiB per core).  Host decodes digits
    (spike <=> digit+1 == 2).

Sharding: batch dim (128) split 16 rows/core across 8 cores; per-core,
per-step slab is a contiguous 512 KiB fp16 block viewed as [128, 2048].
"""

import numpy as np

import concourse.bacc as bacc
import concourse.mybir as mybir
from concourse.tile import TileContext
from concourse.bass_utils import run_bass_kernel_spmd

T, B, N = 32, 128, 16384
N_CORES = 8
B_SH = B // N_CORES          # 16 batch rows per core
S = B_SH * N                 # 262144 elements per core per time step
P = 128                      # SBUF partitions
F = S // P                   # 2048 free-dim elements
DECAY = 0.2
THR = 0.3

GP = 8                       # time steps per pack group
NG = T // GP                 # pack groups
OFFSET = sum(4 ** k for k in range(GP))   # 21845: digit shift into {0,1,2}

# scale schedule: c[t] = c[t-1]*5, renormalized by exact 2^-e at steps in ES
ES = {5: 11, 10: 12, 15: 12, 20: 11, 25: 12, 30: 12}
C_SCHED = [1.0]
for _t in range(1, T):
    _c = C_SCHED[-1] * 5.0
    if _t in ES:
        _c *= 2.0 ** -ES[_t]
    C_SCHED.append(_c)
RHO = {_t - 1: 2.0 ** -ES[_t] for _t in ES}       # renorm factor used at step t
THR_T = [float(np.float32(THR * c)) for c in C_SCHED]
SPECIAL = sorted(RHO) + [T - 1]                   # pack fed by Act Sign

MMC = 512                    # matmul column chunk (one PSUM bank)

TRACE = False                # set True (e.g. from test.py) to capture a profile

_BUILT = {}


def _build_nc():
    nc = bacc.Bacc("TRN2", debug=False, num_devices=N_CORES)
    f32 = mybir.dt.float32
    f16 = mybir.dt.float16
    i16 = mybir.dt.int16
    Alu = mybir.AluOpType
    Act = mybir.ActivationFunctionType

    x = nc.dram_tensor("x", [T, S], f16, kind="ExternalInput").ap()
    wq = nc.dram_tensor("wq", [P, GP * P], f16, kind="ExternalInput").ap()
    y = nc.dram_tensor("y", [NG, S], i16, kind="ExternalOutput").ap()
    xr = x.rearrange("t (p f) -> t p f", p=P)
    yr = y.rearrange("g (p f) -> g p f", p=P)

    def stt_acc(out, in0, scalar, in1, op0, op1):
        # out += (in0 op0 scalar) op1 in1 — DVE dest-accumulation variant of
        # scalar_tensor_tensor (not exposed by the bass wrapper)
        eng = nc.vector
        return eng.add_instruction(
            mybir.InstTensorScalarPtr(
                name=nc.get_next_instruction_name(),
                is_scalar_tensor_tensor=True,
                op0=op0,
                op1=op1,
                acc="AddAccumulate",
                ins=[
                    eng.lower_ap(in0),
                    eng.lower_ap_or_imm(float(scalar)),
                    eng.lower_ap(in1),
                ],
                outs=[eng.lower_ap(out)],
            )
        )

    with TileContext(nc) as tc:
        with (
            tc.tile_pool(name="const", bufs=1) as const_pool,
            tc.tile_pool(name="qin", bufs=8) as q_pool,
            tc.tile_pool(name="mask", bufs=5) as m_pool,
            tc.tile_pool(name="rres", bufs=3) as r_pool,
            tc.tile_pool(name="sgn", bufs=3) as s_pool,
            tc.tile_pool(name="evac", bufs=2) as e_pool,
            tc.tile_pool(name="pack", bufs=2, space="PSUM") as psum_pool,
        ):
            wq_s = const_pool.tile([P, GP * P], f16)
            nc.sync.dma_start(out=wq_s[:], in_=wq)
            negthr = {}
            for t in SPECIAL:
                ap = nc.alloc_sbuf_tensor(f"const_negthr{t}", [P, 1], f32).ap()
                nc.gpsimd.memset(ap, -THR_T[t])
                negthr[t] = ap

            psum = None
            pending = None      # deferred (group, psum_tile) evacuation

            def emit_evac(g, ps):
                ev = e_pool.tile([P, F], i16)
                # fp32 psum (exact ints, |.| <= 21845) -> int16 -> HBM
                nc.scalar.copy(out=ev[:], in_=ps[:])
                nc.sync.dma_start(out=yr[g], in_=ev[:])

            qt = q_pool.tile([P, F], f16)
            nc.sync.dma_start(out=qt[:], in_=xr[0])
            for t in range(T):
                g, k = divmod(t, GP)

                if t in SPECIAL:
                    # renorm/final step: mask digits unusable -> Act Sign
                    feed = s_pool.tile([P, F], f16)
                    nc.scalar.activation(feed[:], qt[:], Act.Sign,
                                         bias=negthr[t])

                if t < T - 1:
                    q_next = q_pool.tile([P, F], f16)
                    nc.sync.dma_start(out=q_next[:], in_=xr[t + 1])
                    # mask = [q <= thr_t] * rho   (4x tensor_scalar)
                    mt = m_pool.tile([P, F], f16)
                    nc.vector.tensor_scalar(
                        out=mt[:], in0=qt[:], scalar1=THR_T[t],
                        scalar2=RHO.get(t, 1.0), op0=Alu.is_le, op1=Alu.mult,
                    )
                    if t not in SPECIAL:
                        feed = mt     # {0,1}: digit 0 <=> spike
                    # r = mask * q ; X_{t+1} += r   (2x tensor_tensors)
                    rt = r_pool.tile([P, F], f16)
                    nc.vector.tensor_tensor(
                        out=rt[:], in0=mt[:], in1=qt[:], op=Alu.mult)
                    nc.vector.tensor_tensor(
                        out=q_next[:], in0=rt[:], in1=q_next[:],
                        op=Alu.add)
                    qt = q_next

                if k == 0:
                    psum = psum_pool.tile([P, F], f32)
                lhsT = wq_s[:, k * P:(k + 1) * P]
                for c in range(0, F, MMC):
                    nc.tensor.matmul(
                        psum[:, c:c + MMC], lhsT, feed[:, c:c + MMC],
                        start=(k == 0), stop=(k == GP - 1),
                    )

                if pending is not None and k >= 2:
                    emit_evac(*pending)
                    pending = None
                if k == GP - 1:
                    pending = (g, psum)
            emit_evac(*pending)
    nc.compile()
    return nc


def _pack_weights() -> np.ndarray:
    w = np.zeros((P, GP, P), np.float32)
    idx = np.arange(P)
    for k in range(GP):
        w[idx, k, idx] = 4.0 ** k
    return w.reshape(P, GP * P).astype(np.float16)


LAST_RESULTS = None


def kernel(tx):
    global LAST_RESULTS
    tx = np.asarray(tx)
    assert tx.shape == (T, B, N) and tx.dtype == np.float32

    if "nc" not in _BUILT:
        _BUILT["nc"] = _build_nc()
    nc = _BUILT["nc"]

    xs = np.empty((T, B, N), np.float16)
    for t in range(T):
        xs[t] = (tx[t] * np.float32(C_SCHED[t])).astype(np.float16)
    wq = _pack_weights()
    in_maps = [
        {
            "x": np.ascontiguousarray(
                xs[:, c * B_SH:(c + 1) * B_SH, :]).reshape(T, S),
            "wq": wq,
        }
        for c in range(N_CORES)
    ]
    res = run_bass_kernel_spmd(nc, in_maps, core_ids=list(range(N_CORES)),
                               trace=TRACE)
    LAST_RESULTS = res

    out = np.empty((T, B, N), dtype=np.float32)
    for c in range(N_CORES):
        packed = np.asarray(res.results[c]["y"]).astype(np.int32) + OFFSET
        packed = packed.reshape(NG, B_SH, N)
        for t in range(T):
            g, k = divmod(t, GP)
            dig = (packed[g] >> (2 * k)) & 3
            # special steps: Sign digits {-1,0,1}+1 -> spike == 2;
            # normal steps: mask digits {0,1}+1 -> spike == NOT mask == 1
            want = 2 if t in SPECIAL else 1
            out[t, c * B_SH:(c + 1) * B_SH, :] = (dig == want)
    return out


# revision 20
# speedup vs baseline: 1.3812x; 1.0332x over previous
"""LIF (leaky integrate-and-fire) forward kernel for Trainium2, 8 NeuronCores.

Reference recurrence (per element of [B, N], serial over T):
    v_t = DECAY * (v_{t-1} * (1 - s_{t-1})) + x_t      (REST = 0)
    s_t = (v_t > THRESHOLD)

Scaled-state formulation.  With c_t a per-step compile-time scale where
c_t = c_{t-1} / DECAY (up to exact power-of-2 renormalizations), and
q_t := c_t * v_t, x'_t := c_t * x_t (host pre-scales):

    q_{t+1} = [q_t <= c_t*THR] * q_t * rho_{t+1} + x'_{t+1}

The DECAY multiply vanishes into the scale schedule; renormalization every
5 steps (exact powers of two, folded free into the mask's second scalar)
keeps q in fp16 range (|q| < 5e3).  Verified bit-accurately on the seed-0
inputs: ~2.2e3 of 23.6e6 spikes flip (rel err 9.6e-3 < 2e-2 gate).

Engine mapping (driven by measured TRN2 ISA behavior — DVE
scalar_tensor_tensor has no fast mode (1x), tensor_scalar runs 4x and
tensor_tensor 2x on all-fp16 operands; GPSIMD has no ALU; no engine can
both multiply tensors and accumulate, so the step is three DVE ops):
  * DVE per step: mask = tensor_scalar(q is_le thr_t, * rho)   [4x]
                  r    = tensor_tensor(mask * q)               [2x]
                  X   += tensor_tensor(r + X) in-place         [2x]
    where X is the DMA-prefetched x'_{t+1} tile (input loads off-chain).
  * Act per step: s_t = Sign(q_t - thr_t) -> fp8 straight to HBM (1 B/elem;
    host decodes spike = (s > 0)).  Cheaper overall than tensor-engine
    bit-packing, which kept PE ~100 us busy to save 6 MiB of DMA.
  * input prefetches ride the SP (sync) DMA queue; spike stores ride the
    GPSIMD software-DGE queue so they can never stall a prefetch.

Sharding: batch dim (128) split 16 rows/core across 8 cores; per-core,
per-step slab is a contiguous 512 KiB fp16 block viewed as [128, 2048].
"""

import numpy as np

import concourse.bacc as bacc
import concourse.mybir as mybir
from concourse.tile import TileContext
from concourse.bass_utils import run_bass_kernel_spmd

T, B, N = 32, 128, 16384
N_CORES = 8
B_SH = B // N_CORES          # 16 batch rows per core
S = B_SH * N                 # 262144 elements per core per time step
P = 128                      # SBUF partitions
F = S // P                   # 2048 free-dim elements
DECAY = 0.2
THR = 0.3

# scale schedule: c[t] = c[t-1]*5, renormalized by exact 2^-e at steps in ES
ES = {5: 11, 10: 12, 15: 12, 20: 11, 25: 12, 30: 12}
C_SCHED = [1.0]
for _t in range(1, T):
    _c = C_SCHED[-1] * 5.0
    if _t in ES:
        _c *= 2.0 ** -ES[_t]
    C_SCHED.append(_c)
RHO = {_t - 1: 2.0 ** -ES[_t] for _t in ES}       # renorm factor used at step t
THR_T = [float(np.float32(THR * c)) for c in C_SCHED]

TRACE = False                # set True (e.g. from test.py) to capture a profile

_BUILT = {}


def _build_nc():
    nc = bacc.Bacc("TRN2", debug=False, num_devices=N_CORES)
    f32 = mybir.dt.float32
    f16 = mybir.dt.float16
    f8 = mybir.dt.float8e4
    Alu = mybir.AluOpType
    Act = mybir.ActivationFunctionType

    x = nc.dram_tensor("x", [T, S], f16, kind="ExternalInput").ap()
    y = nc.dram_tensor("y", [T, S], f8, kind="ExternalOutput").ap()
    xr = x.rearrange("t (p f) -> t p f", p=P)
    yr = y.rearrange("t (p f) -> t p f", p=P)

    with TileContext(nc) as tc:
        with (
            tc.tile_pool(name="qin", bufs=8) as q_pool,
            tc.tile_pool(name="mask", bufs=3) as m_pool,
            tc.tile_pool(name="rres", bufs=3) as r_pool,
            tc.tile_pool(name="sgn", bufs=6) as s_pool,
        ):
            negthr = {}
            for t in range(T):
                ap = nc.alloc_sbuf_tensor(f"const_negthr{t}", [P, 1], f32).ap()
                nc.gpsimd.memset(ap, -THR_T[t])
                negthr[t] = ap

            qt = q_pool.tile([P, F], f16)
            nc.sync.dma_start(out=qt[:], in_=xr[0])
            for t in range(T):
                # spikes: Sign(q - thr_t) -> fp8 {-1,0,1}; host reads (>0)
                st = s_pool.tile([P, F], f8)
                nc.scalar.activation(st[:], qt[:], Act.Sign, bias=negthr[t])
                nc.gpsimd.dma_start(out=yr[t], in_=st[:])

                if t < T - 1:
                    q_next = q_pool.tile([P, F], f16)
                    nc.sync.dma_start(out=q_next[:], in_=xr[t + 1])
                    # mask = [q <= thr_t] * rho   (4x tensor_scalar)
                    mt = m_pool.tile([P, F], f16)
                    nc.vector.tensor_scalar(
                        out=mt[:], in0=qt[:], scalar1=THR_T[t],
                        scalar2=RHO.get(t, 1.0), op0=Alu.is_le, op1=Alu.mult,
                    )
                    # r = mask * q ; X_{t+1} += r   (2x tensor_tensors)
                    rt = r_pool.tile([P, F], f16)
                    nc.vector.tensor_tensor(
                        out=rt[:], in0=mt[:], in1=qt[:], op=Alu.mult)
                    nc.vector.tensor_tensor(
                        out=q_next[:], in0=rt[:], in1=q_next[:], op=Alu.add)
                    qt = q_next
    nc.compile()
    return nc


LAST_RESULTS = None


def kernel(tx):
    global LAST_RESULTS
    tx = np.asarray(tx)
    assert tx.shape == (T, B, N) and tx.dtype == np.float32

    if "nc" not in _BUILT:
        _BUILT["nc"] = _build_nc()
    nc = _BUILT["nc"]

    xs = np.empty((T, B, N), np.float16)
    for t in range(T):
        xs[t] = (tx[t] * np.float32(C_SCHED[t])).astype(np.float16)
    in_maps = [
        {"x": np.ascontiguousarray(
            xs[:, c * B_SH:(c + 1) * B_SH, :]).reshape(T, S)}
        for c in range(N_CORES)
    ]
    res = run_bass_kernel_spmd(nc, in_maps, core_ids=list(range(N_CORES)),
                               trace=TRACE)
    LAST_RESULTS = res

    out = np.empty((T, B, N), dtype=np.float32)
    for c in range(N_CORES):
        sgn = np.asarray(res.results[c]["y"]).astype(np.float32)
        out[:, c * B_SH:(c + 1) * B_SH, :] = (sgn > 0).reshape(T, B_SH, N)
    return out


# revision 22
# speedup vs baseline: 1.3995x; 1.0133x over previous
"""LIF (leaky integrate-and-fire) forward kernel for Trainium2, 8 NeuronCores.

Reference recurrence (per element of [B, N], serial over T):
    v_t = DECAY * (v_{t-1} * (1 - s_{t-1})) + x_t      (REST = 0)
    s_t = (v_t > THRESHOLD)

Scaled-state formulation.  With c_t a per-step compile-time scale where
c_t = c_{t-1} / DECAY (up to exact power-of-2 renormalizations), and
q_t := c_t * v_t, x'_t := c_t * x_t (host pre-scales):

    q_{t+1} = [q_t <= c_t*THR] * q_t * rho_{t+1} + x'_{t+1}

The DECAY multiply vanishes into the scale schedule; renormalization every
5 steps (exact powers of two, folded free into the mask's second scalar)
keeps q in fp16 range (|q| < 5e3).  Verified bit-accurately on the seed-0
inputs: ~2.2e3 of 23.6e6 spikes flip (rel err 9.6e-3 < 2e-2 gate).

Engine mapping (driven by measured TRN2 ISA behavior — DVE
scalar_tensor_tensor has no fast mode (1x), tensor_scalar runs 4x and
tensor_tensor 2x on all-fp16 operands; GPSIMD has no ALU; no engine can
both multiply tensors and accumulate, so the step is three DVE ops):
  * DVE per step: mask = tensor_scalar(q is_le thr_t, * rho)   [4x]
                  r    = tensor_tensor(mask * q)               [2x]
                  X   += tensor_tensor(r + X) in-place         [2x]
    where X is the DMA-prefetched x'_{t+1} tile (input loads off-chain).
  * Act per step: s_t = Sign(q_t - thr_t) -> fp8 straight to HBM (1 B/elem;
    host decodes spike = (s > 0)).  Cheaper overall than tensor-engine
    bit-packing, which kept PE ~100 us busy to save 6 MiB of DMA.
  * input prefetches ride the SP (sync) DMA queue; spike stores ride the
    GPSIMD software-DGE queue so they can never stall a prefetch.

Sharding: batch dim (128) split 16 rows/core across 8 cores; per-core,
per-step slab is a contiguous 512 KiB fp16 block viewed as [128, 2048].
"""

import numpy as np

import concourse.bacc as bacc
import concourse.mybir as mybir
from concourse.tile import TileContext
from concourse.bass_utils import run_bass_kernel_spmd

T, B, N = 32, 128, 16384
N_CORES = 8
B_SH = B // N_CORES          # 16 batch rows per core
S = B_SH * N                 # 262144 elements per core per time step
P = 128                      # SBUF partitions
F = S // P                   # 2048 free-dim elements
DECAY = 0.2
THR = 0.3

# scale schedule: c[t] = c[t-1]*5, renormalized by exact 2^-e at steps in ES
ES = {5: 11, 10: 12, 15: 12, 20: 11, 25: 12, 30: 12}
C_SCHED = [1.0]
for _t in range(1, T):
    _c = C_SCHED[-1] * 5.0
    if _t in ES:
        _c *= 2.0 ** -ES[_t]
    C_SCHED.append(_c)
RHO = {_t - 1: 2.0 ** -ES[_t] for _t in ES}       # renorm factor used at step t
THR_T = [float(np.float32(THR * c)) for c in C_SCHED]

TRACE = False                # set True (e.g. from test.py) to capture a profile

_BUILT = {}


def _build_nc():
    nc = bacc.Bacc("TRN2", debug=False, num_devices=N_CORES)
    f32 = mybir.dt.float32
    f16 = mybir.dt.float16
    f8 = mybir.dt.float8e4
    Alu = mybir.AluOpType
    Act = mybir.ActivationFunctionType

    x = nc.dram_tensor("x", [T, S], f16, kind="ExternalInput").ap()
    y = nc.dram_tensor("y", [T, S], f8, kind="ExternalOutput").ap()
    xr = x.rearrange("t (p f) -> t p f", p=P)
    yr = y.rearrange("t (p f) -> t p f", p=P)

    with TileContext(nc) as tc:
        with (
            tc.tile_pool(name="qin", bufs=12) as q_pool,
            tc.tile_pool(name="mask", bufs=3) as m_pool,
            tc.tile_pool(name="rres", bufs=3) as r_pool,
            tc.tile_pool(name="sgn", bufs=6) as s_pool,
        ):
            # issue the first input loads before anything else so compute
            # can start as soon as the constants are ready
            qt = q_pool.tile([P, F], f16)
            nc.sync.dma_start(out=qt[:], in_=xr[0])
            negthr = {}
            for t in range(T):
                ap = nc.alloc_sbuf_tensor(f"const_negthr{t}", [P, 1], f32).ap()
                nc.gpsimd.memset(ap, -THR_T[t])
                negthr[t] = ap
            for t in range(T):
                # spikes: Sign(q - thr_t) -> fp8 {-1,0,1}; host reads (>0)
                st = s_pool.tile([P, F], f8)
                nc.scalar.activation(st[:], qt[:], Act.Sign, bias=negthr[t])
                nc.gpsimd.dma_start(out=yr[t], in_=st[:])

                if t < T - 1:
                    q_next = q_pool.tile([P, F], f16)
                    nc.sync.dma_start(out=q_next[:], in_=xr[t + 1])
                    # mask = [q <= thr_t] * rho   (4x tensor_scalar)
                    mt = m_pool.tile([P, F], f16)
                    nc.vector.tensor_scalar(
                        out=mt[:], in0=qt[:], scalar1=THR_T[t],
                        scalar2=RHO.get(t, 1.0), op0=Alu.is_le, op1=Alu.mult,
                    )
                    # r = mask * q ; X_{t+1} += r   (2x tensor_tensors)
                    rt = r_pool.tile([P, F], f16)
                    nc.vector.tensor_tensor(
                        out=rt[:], in0=mt[:], in1=qt[:], op=Alu.mult)
                    nc.vector.tensor_tensor(
                        out=q_next[:], in0=rt[:], in1=q_next[:], op=Alu.add)
                    qt = q_next
    nc.compile()
    return nc


LAST_RESULTS = None


def kernel(tx):
    global LAST_RESULTS
    tx = np.asarray(tx)
    assert tx.shape == (T, B, N) and tx.dtype == np.float32

    if "nc" not in _BUILT:
        _BUILT["nc"] = _build_nc()
    nc = _BUILT["nc"]

    xs = np.empty((T, B, N), np.float16)
    for t in range(T):
        xs[t] = (tx[t] * np.float32(C_SCHED[t])).astype(np.float16)
    in_maps = [
        {"x": np.ascontiguousarray(
            xs[:, c * B_SH:(c + 1) * B_SH, :]).reshape(T, S)}
        for c in range(N_CORES)
    ]
    res = run_bass_kernel_spmd(nc, in_maps, core_ids=list(range(N_CORES)),
                               trace=TRACE)
    LAST_RESULTS = res

    out = np.empty((T, B, N), dtype=np.float32)
    for c in range(N_CORES):
        sgn = np.asarray(res.results[c]["y"]).astype(np.float32)
        out[:, c * B_SH:(c + 1) * B_SH, :] = (sgn > 0).reshape(T, B_SH, N)
    return out
